# revision 16
# baseline (speedup 1.0000x reference)
"""Bayesian linear layer on 8 TRN2 NeuronCores.

Computes  out = x @ (mu + softplus(rho) * eps_w).T + (bmu + softplus(brho) * eps_b)
for x [16384, 4096], weights [4096, 4096].

Sharding: 2-way split of the batch dim (N) x 4-way split of out_features.
Each core computes an [8192, 1024] fp32 output shard:
  - weight shard W^T is generated on-device: softplus via Exp + Ln(x+1) on the
    ACT engine (table sets batched), FMA on DVE with fp16 output, staged
    through a DRAM scratch, then xbar transpose-loaded as 64 resident tiles
    [128 (in_f), 512 (out_f)] fp16.
  - x is shipped fp16 and xbar transpose-loaded straight from DRAM in
    [1024 x 128] panels (4 k-quarter tiles per 1024-row super-tile).
  - matmuls are fp16, N=512, fp32 PSUM accumulation over 32 k-blocks; the two
    output halves (q=0/1) run as separate phases over 8 PSUM banks so the
    first phase only needs half the prepped weights; bias is added during the
    PSUM->SBUF copy on DVE.
All DMAs stay on the SP HWDGE ring: splitting across the SP+ACT rings
corrupts results on this stack (completion tracking assumes one ring).
"""

import numpy as np

import bass_rust as _bass_rust
import concourse.bacc as bacc
import concourse.tile as tile
from concourse import mybir
from concourse import bass_utils
from concourse.hw_specs import get_activation_tables


class _Bacc(bacc.Bacc):
    """Bacc whose activation-table placement resolves Exp and Ln to the one
    table set containing both (natural_log_exp_and_others), instead of
    thrashing between per-function sets (one 1.3us ACT_TABLE_LOAD per
    ACTIVATE).  List order/indices are preserved -- act_func_set_id is the
    index into act_info.json -- only the membership used for matching is
    restricted."""

    def insert_act_table_loads(self):
        tables = list(get_activation_tables(self.m.arch).items())
        AF = mybir.ActivationFunctionType
        filtered = []
        for name, funcs in tables:
            if name != "natural_log_exp_and_others":
                funcs = funcs - {AF.Exp, AF.Ln}
            filtered.append((name, funcs))
        _bass_rust.insert_act_table_loads(self, filtered)

R, C = 2, 4                      # grid: R-way split of N, C-way split of out_f
N, IN_F, OUT_F = 16384, 4096, 4096
NS, OS = N // R, OUT_F // C      # per-core shards: 8192 rows, 1024 out cols
KB = IN_F // 128                 # 32 k-blocks
NB = 1024                       # rows per super-tile
NKQ = 4                          # k-quarters per super-tile
KQ = KB // NKQ                   # 8 k-blocks per quarter
N_CORES = 8

FP32 = mybir.dt.float32
F16 = mybir.dt.float16


def _build_nc():
    nc = _Bacc("TRN2", target_bir_lowering=False, debug=False)

    xb = nc.dram_tensor("xb", [NS, IN_F], F16, kind="ExternalInput").ap()
    mu = nc.dram_tensor("mu", [OS, IN_F], F16, kind="ExternalInput").ap()
    rho = nc.dram_tensor("rho", [OS, IN_F], F16, kind="ExternalInput").ap()
    eps = nc.dram_tensor("eps", [OS, IN_F], F16, kind="ExternalInput").ap()
    bmu = nc.dram_tensor("bmu", [128, OS], FP32, kind="ExternalInput").ap()
    brho = nc.dram_tensor("brho", [128, OS], FP32, kind="ExternalInput").ap()
    beps = nc.dram_tensor("beps", [128, OS], FP32, kind="ExternalInput").ap()
    out = nc.dram_tensor("out", [NS, OS], FP32, kind="ExternalOutput").ap()

    AF = mybir.ActivationFunctionType
    n_super = NS // NB
    subs = NB // 128

    with tile.TileContext(nc) as tc:
        with (
            tc.tile_pool(name="wt", bufs=1) as wt_pool,
            tc.tile_pool(name="bias", bufs=1) as bias_pool,
            tc.tile_pool(name="prep_rho", bufs=2) as prep_rho,
            tc.tile_pool(name="prep_in", bufs=2) as prep_in,
            tc.tile_pool(name="prep_w", bufs=2) as prep_w,
            tc.tile_pool(name="w16", bufs=1, space="DRAM") as w16_pool,
            tc.tile_pool(name="xt", bufs=1) as xt_pool,
            tc.tile_pool(name="outp", bufs=3) as out_pool,
            tc.tile_pool(name="psum", bufs=1, space="PSUM") as psum_pool,
        ):
            # ---- bias: b = bmu + softplus(brho) * beps, replicated [128, OS]
            bmu_t = bias_pool.tile([128, OS], FP32, tag="bmu")
            brho_t = bias_pool.tile([128, OS], FP32, tag="brho")
            beps_t = bias_pool.tile([128, OS], FP32, tag="beps")
            nc.sync.dma_start(bmu_t[:], bmu[:])
            nc.sync.dma_start(brho_t[:], brho[:])
            nc.sync.dma_start(beps_t[:], beps[:])
            nc.scalar.activation(brho_t[:], brho_t[:], AF.Exp)
            nc.scalar.activation(brho_t[:], brho_t[:], AF.Ln, bias=1.0)
            nc.vector.tensor_mul(beps_t[:], brho_t[:], beps_t[:])
            bias_t = bias_pool.tile([128, OS], FP32, tag="bias")
            nc.vector.tensor_add(bias_t[:], beps_t[:], bmu_t[:])

            # ---- W^T: computed in [o, i] layout, staged to a DRAM scratch
            # (one tile per i-chunk), then transpose-loaded into 32 resident
            # [128, 1024] tiles as each i-chunk completes.
            wts = [wt_pool.tile([128, OS], F16, tag=f"wt{ib}",
                                name=f"wt{ib}") for ib in range(KB)]

            IC = 1024
            NIC = IN_F // IC
            w16 = w16_pool.tile([OS, IN_F], F16, tag="w16", name="w16")

            def prep_chunk(ob, ic):
                # softplus(rho) in place (Exp then Ln(x+1); both live in the
                # single loaded table set), then w = mu + sp*eps -> fp16.
                rho_c = prep_rho.tile([128, IC], F16, tag=f"rho{ob % 4}",
                                      name=f"rho_{ob}_{ic}")
                sl = (slice(ob * 128, (ob + 1) * 128),
                      slice(ic * IC, (ic + 1) * IC))
                nc.sync.dma_start(rho_c[:], rho[sl])
                nc.scalar.activation(rho_c[:], rho_c[:], AF.Exp)
                nc.scalar.activation(rho_c[:], rho_c[:], AF.Ln, bias=1.0)
                mu_c = prep_in.tile([128, IC], F16, tag="mu")
                eps_c = prep_in.tile([128, IC], F16, tag="eps")
                nc.sync.dma_start(mu_c[:], mu[sl])
                nc.sync.dma_start(eps_c[:], eps[sl])
                t32 = prep_w.tile([128, IC], FP32, tag="t32")
                nc.vector.tensor_mul(t32[:], rho_c[:], eps_c[:])
                wf = prep_w.tile([128, IC], F16, tag="wf")
                nc.vector.tensor_add(wf[:], t32[:], mu_c[:])
                nc.sync.dma_start(
                    w16[ob * 128:(ob + 1) * 128, ic * IC:(ic + 1) * IC], wf[:])

            def xt_panel(s, kq):
                xtt = xt_pool.tile([128, KQ * NB], F16, tag=f"kq{kq}",
                                   name=f"xt_s{s}_k{kq}")
                for j in range(KQ):
                    ib = kq * KQ + j
                    nc.sync.dma_start(
                        xtt[:, j * NB:(j + 1) * NB],
                        xb[s * NB:(s + 1) * NB, ib * 128:(ib + 1) * 128],
                        transpose=True)
                return xtt

            def xt_panels(s):
                return [xt_panel(s, kq) for kq in range(NKQ)]

            # emission order: weight prep with the first super-tile's x
            # panels interleaved (panels fill ring idle during the prep dep
            # chain), then the 32 big weight transpose-loads.  The single
            # w16 tile makes every wts wait for the whole prep: the PE then
            # starts once, stays dense, and never re-throttles.
            xtq0 = []
            for ic in range(NIC):
                for ob in range(OS // 128):
                    prep_chunk(ob, ic)
                xtq0.append(xt_panel(0, ic))
            for ib in range(KB):
                nc.sync.dma_start(wts[ib][:],
                                  w16[:, ib * 128:(ib + 1) * 128],
                                  transpose=True)

            # ---- main loop
            for s in range(n_super):
                xtq = xtq0 if s == 0 else xt_panels(s)
                for q in range(2):
                    psq = [psum_pool.tile([128, 512], FP32, tag=f"ps{sub}",
                                          name=f"ps_{s}_{q}_{sub}")
                           for sub in range(subs)]
                    for kq in range(NKQ):
                        for sub in range(subs):
                            for j in range(KQ):
                                ib = kq * KQ + j
                                xs = xtq[kq][:, j * NB + sub * 128:
                                             j * NB + (sub + 1) * 128]
                                nc.tensor.matmul(
                                    psq[sub][:], xs,
                                    wts[ib][:, q * 512:(q + 1) * 512],
                                    start=(ib == 0), stop=(ib == KB - 1))
                    for sub in range(subs):
                        ot = out_pool.tile([128, 512], FP32, tag="ot",
                                           name=f"ot_{s}_{q}_{sub}")
                        nc.vector.tensor_add(
                            ot[:], psq[sub][:], bias_t[:, q * 512:(q + 1) * 512])
                        row = (s * subs + sub) * 128
                        nc.sync.dma_start(
                            out[row:row + 128, q * 512:(q + 1) * 512], ot[:])

    nc.compile()
    return nc


_NC = None


def _get_nc():
    global _NC
    if _NC is None:
        _NC = _build_nc()
    return _NC


def kernel(x, weight_mu, weight_rho, bias_mu, bias_rho, eps_w, eps_b,
           _trace=False, _trace_kwargs=None):
    x = np.asarray(x, dtype=np.float32)
    weight_mu = np.asarray(weight_mu, dtype=np.float32)
    weight_rho = np.asarray(weight_rho, dtype=np.float32)
    bias_mu = np.asarray(bias_mu, dtype=np.float32)
    bias_rho = np.asarray(bias_rho, dtype=np.float32)
    eps_w = np.asarray(eps_w, dtype=np.float32)
    eps_b = np.asarray(eps_b, dtype=np.float32)

    nc = _get_nc()
    xb = x.astype(np.float16)

    in_maps = []
    for c in range(N_CORES):
        r, q = divmod(c, C)
        osl = slice(q * OS, (q + 1) * OS)
        in_maps.append({
            "xb": xb[r * NS:(r + 1) * NS],
            "mu": weight_mu[osl].astype(np.float16),
            "rho": weight_rho[osl].astype(np.float16),
            "eps": eps_w[osl].astype(np.float16),
            "bmu": np.ascontiguousarray(np.broadcast_to(bias_mu[osl], (128, OS))),
            "brho": np.ascontiguousarray(np.broadcast_to(bias_rho[osl], (128, OS))),
            "beps": np.ascontiguousarray(np.broadcast_to(eps_b[osl], (128, OS))),
        })

    kwargs = {}
    if _trace:
        kwargs["trace"] = True
        if _trace_kwargs:
            kwargs.update(_trace_kwargs)
    res = bass_utils.run_bass_kernel_spmd(
        nc, in_maps, core_ids=list(range(N_CORES)), **kwargs)

    out = np.empty((N, OUT_F), np.float32)
    for c in range(N_CORES):
        r, q = divmod(c, C)
        out[r * NS:(r + 1) * NS, q * OS:(q + 1) * OS] = res.results[c]["out"]
    if _trace:
        return out, res
    return out
